# revision 22
# baseline (speedup 1.0000x reference)
"""Trainium2 Bass kernel for nn_ModelNew_17411797418162.

Computation (per (b,s) sample):
  mixed = h_res @ x            # [4,4] @ [4,1024]
  out   = mixed * h_out[None,:] + h_post[:,None] * x

Sharding: pure data parallel over the leading batch dim B=8 -> 1 batch/core.

Per-core design (memory-bound). The rel-err gate (2e-2) leaves huge
precision headroom over fp32, so all HBM traffic is fp16 (measured rel
err 7.2e-4 end to end on HW): x in 16MB + out 16MB + h_out 4MB = 36MB of
streaming traffic per core per pass (+2MB resident block-diag weights,
loaded once), vs 72MB streamed by the fp32 baseline.

x / out / h_out are pre-permuted HOST-SIDE (layout only) into
partition-major order (tensor[p, blk, d] = flat[128*blk + p, d]) so every
streaming DMA moves long (8KB) per-partition contiguous runs instead of
interleaved 2KB rows - measured DMA-only floor 116us vs 130us for the
row-interleaved layout (310 GB/s/core effective).

Math structure (minimizes per-block PSUM round-trips; each PSUM-sourced
vector op costs ~1.2us at 1x rate - measured):
  out = h_res @ (x * bcast(h_out)) + diag(h_post) @ x
- bcast(h_out) onto the 4 stream rows runs on the PE: a constant 0/1
  fp16 matrix E (lhsT, K=32 via tile_position) replicates 32 samples'
  h_out rows to 128 (sample,stream) partitions in PSUM (psh).
- DVE: y = x * psh (one PSUM-sourced tensor_tensor per blk, 1.2us).
- PE: ps = W_blk.T @ y + DG_blk.T @ x accumulated in one PSUM tile.
  W = block-diag scatter of h_res^T (host-side, layout only);
  DG = diag(h_post), built on-chip once (identity * h_post column).
- ACT: one PSUM-sourced copy evacuates ps -> out_sb fp16 (1.23us).
Per 1MB chunk (4 sub-blocks): all 4 broadcasts are emitted up front
(filling the shared 4-buf PSUM ring) so the in-order PE never
head-of-line blocks on the DVE; then 4x [y, mixed matmuls, evac]; store.

Engine budget/core (measured): DMA 116us floor, DVE 77us, ACT 79us,
PE ~110-130us (per-MM cost ~2x the doc model due to PSUM-bank sem
waits / HAM micro-oscillation at N=512 - dtype-independent, micro-
benchmarked). Measured end-to-end: ~135-137us/pass (vs 270us baseline).
Structures tried and rejected (all measured slower): 2MB chunks (200us),
flat 1-ahead pipeline with split PSUM rings (150us), 0.5MB chunks
(143-151us), ACT-assisted h4 hybrid (139-193us), fused DVE
scalar_tensor_tensor epilogue (164us).
"""
import numpy as np

import concourse.bacc as bacc
import concourse.tile as tile
import concourse.mybir as mybir
from concourse.bass_utils import run_bass_kernel_spmd

B, S, N, D = 8, 2048, 4, 1024
NCORES = 8
ROWS = S * N              # 8192 flattened rows per core
NSB = 16                  # streaming chunks (512 rows / 1MB) per core
SUBS = 4                  # sub-blocks (32 samples / 128 rows) per chunk
NBLK = 64                 # total sub-blocks per core
F32 = mybir.dt.float32
FP16 = mybir.dt.float16

_cache = {}


def build_program(iters: int = 1, mode: str = "full"):
    """Build the SPMD Bass program (one core's view). Cached per (iters, mode).

    mode: "full" = real kernel; ablations for bottleneck isolation:
      "dma"   = loads + store only (wrong output values)
      "nodve" = skip the y multiply, feed x to both matmuls (wrong values)
      "noact" = skip evacuation + store x_t instead (wrong values)
    """
    if (iters, mode) in _cache:
        return _cache[(iters, mode)]

    nc = bacc.Bacc("TRN2", target_bir_lowering=False, debug=False)
    x = nc.dram_tensor("x", [ROWS, D], FP16, kind="ExternalInput")
    w = nc.dram_tensor("w", [NBLK, 128, 128], FP16, kind="ExternalInput")
    ho = nc.dram_tensor("ho", [S, D], FP16, kind="ExternalInput")
    e4 = nc.dram_tensor("e4", [128, 128], FP16, kind="ExternalInput")
    hp = nc.dram_tensor("hp", [128, NBLK], F32, kind="ExternalInput")
    ident = nc.dram_tensor("ident", [128, 128], FP16, kind="ExternalInput")
    out = nc.dram_tensor("out", [ROWS, D], FP16, kind="ExternalOutput")

    with tile.TileContext(nc) as tc:
        with (
            tc.tile_pool(name="const", bufs=1) as cpool,
            tc.tile_pool(name="big", bufs=3) as bpool,
            tc.tile_pool(name="hop", bufs=2) as hpool,
            tc.tile_pool(name="mid", bufs=3) as mpool,
            tc.tile_pool(name="psum", bufs=4, space="PSUM") as ppool,
        ):
            e4_t = cpool.tile([128, 128], FP16)
            nc.gpsimd.dma_start(e4_t[:], e4.ap())
            # resident block-diag weights: w_all[r, (b, c)] = w[b, r, c]
            w_all = cpool.tile([128, NBLK * 128], FP16)
            nc.gpsimd.dma_start(
                w_all[:].rearrange("r (b c) -> r b c", b=NBLK),
                w.ap().rearrange("b r c -> r b c"))
            # diag(h_post) built on-chip: DG_b = identity * h_post[:, b]
            hp_t = cpool.tile([128, NBLK], F32)
            nc.gpsimd.dma_start(hp_t[:], hp.ap())
            id_t = cpool.tile([128, 128], FP16)
            nc.gpsimd.dma_start(id_t[:], ident.ap())
            dg_all = cpool.tile([128, NBLK * 128], FP16)
            for b in range(NBLK):
                nc.vector.tensor_scalar_mul(
                    dg_all[:, 128 * b:128 * (b + 1)], id_t[:],
                    hp_t[:, b:b + 1])

            def body():
                ho_g = None

                def bcast(blk):
                    # replicate h_out rows of sub-block blk onto the 4
                    # stream partitions via constant 0/1 matmul
                    q = 32 * (blk % 4)
                    g = (blk // 4) % 4  # ho_g holds 4 sample-chunks
                    psh = ppool.tile([128, D], F32, tag="ps")
                    for c in range(2):
                        nc.tensor.matmul(
                            psh[:, 512 * c:512 * (c + 1)],
                            e4_t[q:q + 32, :],
                            ho_g[q:q + 32,
                                 D * g + 512 * c:D * g + 512 * (c + 1)],
                            start=True, stop=True,
                            tile_position=(q, 0))
                    return psh

                for sb in range(NSB):
                    if sb % 4 == 0:
                        # h_out rows for 4 chunks (512 samples), one 1MB
                        # DMA, 8KB contiguous per partition
                        ho_g = hpool.tile([128, 4 * D], FP16, tag="hog")
                        nc.gpsimd.dma_start(
                            ho_g[:].rearrange("p (g d) -> p g d", g=4),
                            ho.ap()[128 * sb:128 * (sb + 4), :].rearrange(
                                "(g p) d -> p g d", g=4))

                    # x sub-blocks: 8KB contiguous per partition
                    x_t = bpool.tile([128, SUBS * D], FP16, tag="x")
                    nc.sync.dma_start(
                        x_t[:].rearrange("p (k d) -> p k d", k=SUBS),
                        x.ap()[512 * sb:512 * (sb + 1), :].rearrange(
                            "(k p) d -> p k d", k=SUBS))
                    if mode in ("full", "nodve"):
                        out_sb = bpool.tile([128, SUBS * D], FP16,
                                            tag="out")

                    # all SUBS broadcasts up front (fills the shared PSUM
                    # ring) so the PE never head-of-line blocks on the DVE
                    pshs = []
                    if mode != "dma":
                        pshs = [bcast(SUBS * sb + k) for k in range(SUBS)]

                    for k in range(SUBS):
                        blk = SUBS * sb + k
                        xk = x_t[:, D * k:D * (k + 1)]
                        if mode == "dma":
                            continue
                        # y = x * bcast(h_out)
                        if mode == "full":
                            y_t = mpool.tile([128, D], FP16, tag="y")
                            nc.vector.tensor_mul(y_t[:], xk, pshs[k][:])
                            yk = y_t[:]
                        else:
                            yk = xk

                        # ps = W.T @ y + DG.T @ x (fp32 PSUM accum);
                        # W for both halves then DG for both halves:
                        # 2 weight loads per blk instead of 4
                        ps = ppool.tile([128, D], F32, tag="ps")
                        lhsW = w_all[:, 128 * blk:128 * (blk + 1)]
                        lhsD = dg_all[:, 128 * blk:128 * (blk + 1)]
                        for c in range(2):
                            sl = slice(512 * c, 512 * (c + 1))
                            nc.tensor.matmul(ps[:, sl], lhsW, yk[:, sl],
                                             start=True, stop=False)
                        for c in range(2):
                            sl = slice(512 * c, 512 * (c + 1))
                            nc.tensor.matmul(ps[:, sl], lhsD, xk[:, sl],
                                             start=False, stop=True)
                        # evacuate (converts f32 -> fp16)
                        if mode != "noact":
                            nc.scalar.copy(
                                out_sb[:, D * k:D * (k + 1)], ps[:])

                    src_sb = out_sb if mode in ("full", "nodve") else x_t
                    nc.scalar.dma_start(
                        out.ap()[512 * sb:512 * (sb + 1), :].rearrange(
                            "(k p) d -> p k d", k=SUBS),
                        src_sb[:].rearrange("p (k d) -> p k d", k=SUBS))

            if iters == 1:
                body()
            else:
                with tc.For_i(0, iters, 1):
                    body()

    nc.compile()
    _cache[(iters, mode)] = nc
    return nc


def make_in_maps(x, h_res, h_out, h_post):
    """Split full inputs into per-core input maps (host-side, layout +
    dtype-conversion only)."""
    x = np.ascontiguousarray(x, dtype=np.float32)
    h_res = np.ascontiguousarray(h_res, dtype=np.float32)
    h_out = np.ascontiguousarray(h_out, dtype=np.float32)
    h_post = np.ascontiguousarray(h_post, dtype=np.float32)

    # stream-replication matrix: e4[32k + q, 4q + i] = 1.0
    e4 = np.zeros((128, 128), np.float16)
    q = np.arange(128)
    for i in range(4):
        e4[q, 4 * (q % 32) + i] = 1.0
    ident = np.eye(128, dtype=np.float16)

    idx = np.arange(32)
    in_maps = []
    for c in range(NCORES):
        xc = x[c].reshape(ROWS, D).astype(np.float16)
        # Block-diagonal weights: W[b, 4p+j, 4p+i] = h_res[c, 32b+p, i, j]
        hr = h_res[c].reshape(NBLK, 32, 4, 4)            # [b, p, i, j]
        Wb = np.zeros((NBLK, 32, 4, 32, 4), np.float16)  # [b, (p,j), (p,i)]
        Wb[:, idx, :, idx, :] = hr.transpose(1, 0, 3, 2).astype(np.float16)
        hoc = h_out[c].astype(np.float16)
        # h_post columns: hp[r, b] = h_post_flat[128b + r]
        hpc = np.ascontiguousarray(
            h_post[c].reshape(NBLK, 128).T).astype(np.float32)
        in_maps.append({
            "x": xc,
            "w": Wb.reshape(NBLK, 128, 128),
            "ho": hoc,
            "e4": e4,
            "hp": hpc,
            "ident": ident,
        })
    return in_maps


def kernel(x, h_res, h_out, h_post):
    nc = build_program(iters=1)
    in_maps = make_in_maps(x, h_res, h_out, h_post)
    res = run_bass_kernel_spmd(nc, in_maps, list(range(NCORES)))
    out = np.stack([res.results[c]["out"].astype(np.float32)
                    .reshape(S, N, D)
                    for c in range(NCORES)])
    return out


# revision 24
# speedup vs baseline: 1.1208x; 1.1208x over previous
"""Trainium2 Bass kernel for nn_ModelNew_17411797418162.

Computation (per (b,s) sample):
  mixed = h_res @ x            # [4,4] @ [4,1024]
  out   = mixed * h_out[None,:] + h_post[:,None] * x

Sharding: pure data parallel over the leading batch dim B=8 -> 1 batch/core.

Per-core design (memory-bound). The rel-err gate (2e-2) leaves huge
precision headroom over fp32, so all HBM traffic is fp16 (measured rel
err 7.2e-4 end to end on HW): x in 16MB + out 16MB + h_out 4MB = 36MB of
streaming traffic per core per pass (+2MB resident block-diag weights,
loaded once), vs 72MB streamed by the fp32 baseline.

x / out / h_out are pre-permuted HOST-SIDE (layout only) into
partition-major order (tensor[p, blk, d] = flat[128*blk + p, d]) so every
streaming DMA moves long (8KB) per-partition contiguous runs instead of
interleaved 2KB rows - measured DMA-only floor 116us vs 130us for the
row-interleaved layout (310 GB/s/core effective).

Math structure (minimizes per-block PSUM round-trips; each PSUM-sourced
vector op costs ~1.2us at 1x rate - measured):
  out = h_res @ (x * bcast(h_out)) + diag(h_post) @ x
- bcast(h_out) onto the 4 stream rows runs on the PE: a constant 0/1
  fp16 matrix E (lhsT, K=32 via tile_position) replicates 32 samples'
  h_out rows to 128 (sample,stream) partitions in PSUM (psh).
- DVE: y = x * psh (one PSUM-sourced tensor_tensor per blk, 1.2us).
- PE: ps = W_blk.T @ y + DG_blk.T @ x accumulated in one PSUM tile.
  W = block-diag scatter of h_res^T (host-side, layout only);
  DG = diag(h_post), built on-chip once (identity * h_post column).
- ACT: one PSUM-sourced copy evacuates ps -> out_sb fp16 (1.23us).
Per 1MB chunk (4 sub-blocks): all 4 broadcasts are emitted up front
(filling the shared 4-buf PSUM ring) so the in-order PE never
head-of-line blocks on the DVE; then 4x [y, mixed matmuls, evac]; store.

Engine budget/core (measured): DMA 116us floor, DVE 77us, ACT 79us,
PE ~110-130us (per-MM cost ~2x the doc model due to PSUM-bank sem
waits / HAM micro-oscillation at N=512 - dtype-independent, micro-
benchmarked). Measured end-to-end: ~135-137us/pass (vs 270us baseline).
Structures tried and rejected (all measured slower): 2MB chunks (200us),
flat 1-ahead pipeline with split PSUM rings (150us), 0.5MB chunks
(143-151us), ACT-assisted h4 hybrid (139-193us), fused DVE
scalar_tensor_tensor epilogue (164us), row-interleaved DMA layout
re-test (154us).
"""
import numpy as np

import concourse.bacc as bacc
import concourse.tile as tile
import concourse.mybir as mybir
from concourse.bass_utils import run_bass_kernel_spmd

B, S, N, D = 8, 2048, 4, 1024
NCORES = 8
ROWS = S * N              # 8192 flattened rows per core
NSB = 16                  # streaming chunks (512 rows / 1MB) per core
SUBS = 4                  # sub-blocks (32 samples / 128 rows) per chunk
NBLK = 64                 # total sub-blocks per core
F32 = mybir.dt.float32
FP16 = mybir.dt.float16

_cache = {}


def build_program(iters: int = 1, mode: str = "full"):
    """Build the SPMD Bass program (one core's view). Cached per (iters, mode).

    mode: "full" = real kernel; ablations for bottleneck isolation:
      "dma"   = loads + store only (wrong output values)
      "nodve" = skip the y multiply, feed x to both matmuls (wrong values)
      "noact" = skip evacuation + store x_t instead (wrong values)
    """
    if (iters, mode) in _cache:
        return _cache[(iters, mode)]

    nc = bacc.Bacc("TRN2", target_bir_lowering=False, debug=False)
    # partition-major layouts: [p, blk, d] = row-major flat[128*blk + p, d]
    x = nc.dram_tensor("x", [128, NBLK, D], FP16, kind="ExternalInput")
    w = nc.dram_tensor("w", [NBLK, 128, 128], FP16, kind="ExternalInput")
    ho = nc.dram_tensor("ho", [128, S // 128, D], FP16, kind="ExternalInput")
    e4 = nc.dram_tensor("e4", [128, 128], FP16, kind="ExternalInput")
    hp = nc.dram_tensor("hp", [128, NBLK], F32, kind="ExternalInput")
    ident = nc.dram_tensor("ident", [128, 128], FP16, kind="ExternalInput")
    out = nc.dram_tensor("out", [128, NBLK, D], FP16, kind="ExternalOutput")

    with tile.TileContext(nc) as tc:
        with (
            tc.tile_pool(name="const", bufs=1) as cpool,
            tc.tile_pool(name="big", bufs=3) as bpool,
            tc.tile_pool(name="hop", bufs=2) as hpool,
            tc.tile_pool(name="mid", bufs=3) as mpool,
            tc.tile_pool(name="psum", bufs=4, space="PSUM") as ppool,
        ):
            e4_t = cpool.tile([128, 128], FP16)
            nc.gpsimd.dma_start(e4_t[:], e4.ap())
            # resident block-diag weights: w_all[r, (b, c)] = w[b, r, c]
            w_all = cpool.tile([128, NBLK * 128], FP16)
            nc.gpsimd.dma_start(
                w_all[:].rearrange("r (b c) -> r b c", b=NBLK),
                w.ap().rearrange("b r c -> r b c"))
            # diag(h_post) built on-chip: DG_b = identity * h_post[:, b]
            hp_t = cpool.tile([128, NBLK], F32)
            nc.gpsimd.dma_start(hp_t[:], hp.ap())
            id_t = cpool.tile([128, 128], FP16)
            nc.gpsimd.dma_start(id_t[:], ident.ap())
            dg_all = cpool.tile([128, NBLK * 128], FP16)
            for b in range(NBLK):
                nc.vector.tensor_scalar_mul(
                    dg_all[:, 128 * b:128 * (b + 1)], id_t[:],
                    hp_t[:, b:b + 1])

            def body():
                ho_g = None

                def bcast(blk):
                    # replicate h_out rows of sub-block blk onto the 4
                    # stream partitions via constant 0/1 matmul
                    q = 32 * (blk % 4)
                    g = (blk // 4) % 4  # ho_g holds 4 sample-chunks
                    psh = ppool.tile([128, D], F32, tag="ps")
                    for c in range(2):
                        nc.tensor.matmul(
                            psh[:, 512 * c:512 * (c + 1)],
                            e4_t[q:q + 32, :],
                            ho_g[q:q + 32,
                                 D * g + 512 * c:D * g + 512 * (c + 1)],
                            start=True, stop=True,
                            tile_position=(q, 0))
                    return psh

                for sb in range(NSB):
                    if sb % 4 == 0:
                        # h_out rows for 4 chunks (512 samples), one 1MB
                        # DMA, 8KB contiguous per partition
                        ho_g = hpool.tile([128, 4 * D], FP16, tag="hog")
                        nc.gpsimd.dma_start(
                            ho_g[:].rearrange("p (g d) -> p g d", g=4),
                            ho.ap()[:, sb:sb + 4, :])

                    # x sub-blocks: 8KB contiguous per partition
                    x_t = bpool.tile([128, SUBS * D], FP16, tag="x")
                    nc.sync.dma_start(
                        x_t[:].rearrange("p (k d) -> p k d", k=SUBS),
                        x.ap()[:, SUBS * sb:SUBS * (sb + 1), :])
                    if mode in ("full", "nodve"):
                        out_sb = bpool.tile([128, SUBS * D], FP16,
                                            tag="out")

                    # all SUBS broadcasts up front (fills the shared PSUM
                    # ring) so the PE never head-of-line blocks on the DVE
                    pshs = []
                    if mode != "dma":
                        pshs = [bcast(SUBS * sb + k) for k in range(SUBS)]

                    for k in range(SUBS):
                        blk = SUBS * sb + k
                        xk = x_t[:, D * k:D * (k + 1)]
                        if mode == "dma":
                            continue
                        # y = x * bcast(h_out)
                        if mode == "full":
                            y_t = mpool.tile([128, D], FP16, tag="y")
                            nc.vector.tensor_mul(y_t[:], xk, pshs[k][:])
                            yk = y_t[:]
                        else:
                            yk = xk

                        # ps = W.T @ y + DG.T @ x (fp32 PSUM accum);
                        # alternating W,DG per half-bank (the order the
                        # best-measured variant used - accumulation pairs
                        # complete a region back-to-back)
                        ps = ppool.tile([128, D], F32, tag="ps")
                        lhsW = w_all[:, 128 * blk:128 * (blk + 1)]
                        lhsD = dg_all[:, 128 * blk:128 * (blk + 1)]
                        for c in range(2):
                            sl = slice(512 * c, 512 * (c + 1))
                            nc.tensor.matmul(ps[:, sl], lhsW, yk[:, sl],
                                             start=True, stop=False)
                            nc.tensor.matmul(ps[:, sl], lhsD, xk[:, sl],
                                             start=False, stop=True)
                        # evacuate (converts f32 -> fp16)
                        if mode != "noact":
                            nc.scalar.copy(
                                out_sb[:, D * k:D * (k + 1)], ps[:])

                    src_sb = out_sb if mode in ("full", "nodve") else x_t
                    nc.scalar.dma_start(
                        out.ap()[:, SUBS * sb:SUBS * (sb + 1), :],
                        src_sb[:].rearrange("p (k d) -> p k d", k=SUBS))

            if iters == 1:
                body()
            else:
                with tc.For_i(0, iters, 1):
                    body()

    nc.compile()
    _cache[(iters, mode)] = nc
    return nc


def make_in_maps(x, h_res, h_out, h_post):
    """Split full inputs into per-core input maps (host-side, layout +
    dtype-conversion only)."""
    x = np.ascontiguousarray(x, dtype=np.float32)
    h_res = np.ascontiguousarray(h_res, dtype=np.float32)
    h_out = np.ascontiguousarray(h_out, dtype=np.float32)
    h_post = np.ascontiguousarray(h_post, dtype=np.float32)

    # stream-replication matrix: e4[32k + q, 4q + i] = 1.0
    e4 = np.zeros((128, 128), np.float16)
    q = np.arange(128)
    for i in range(4):
        e4[q, 4 * (q % 32) + i] = 1.0
    ident = np.eye(128, dtype=np.float16)

    idx = np.arange(32)
    in_maps = []
    for c in range(NCORES):
        # partition-major: x[p, blk, d] = x_flat[128*blk + p, d]
        xc = np.ascontiguousarray(
            x[c].reshape(NBLK, 128, D).transpose(1, 0, 2)).astype(np.float16)
        # Block-diagonal weights: W[b, 4p+j, 4p+i] = h_res[c, 32b+p, i, j]
        hr = h_res[c].reshape(NBLK, 32, 4, 4)            # [b, p, i, j]
        Wb = np.zeros((NBLK, 32, 4, 32, 4), np.float16)  # [b, (p,j), (p,i)]
        Wb[:, idx, :, idx, :] = hr.transpose(1, 0, 3, 2).astype(np.float16)
        # h_out partition-major: ho[p, g, d] = h_out[c, 128g + p, d]
        hoc = np.ascontiguousarray(
            h_out[c].reshape(S // 128, 128, D).transpose(1, 0, 2)
        ).astype(np.float16)
        # h_post columns: hp[r, b] = h_post_flat[128b + r]
        hpc = np.ascontiguousarray(
            h_post[c].reshape(NBLK, 128).T).astype(np.float32)
        in_maps.append({
            "x": xc,
            "w": Wb.reshape(NBLK, 128, 128),
            "ho": hoc,
            "e4": e4,
            "hp": hpc,
            "ident": ident,
        })
    return in_maps


def kernel(x, h_res, h_out, h_post):
    nc = build_program(iters=1)
    in_maps = make_in_maps(x, h_res, h_out, h_post)
    res = run_bass_kernel_spmd(nc, in_maps, list(range(NCORES)))
    # un-permute: out[p, blk, d] -> flat[128*blk + p, d]
    out = np.stack([
        res.results[c]["out"].astype(np.float32).transpose(1, 0, 2)
        .reshape(S, N, D)
        for c in range(NCORES)])
    return out
